# revision 1
# baseline (speedup 1.0000x reference)
"""DPLSTMCell Trainium2 kernel.

Data-parallel LSTM cell over 8 NeuronCores: batch dim of input/h_prev/c_prev
is sharded, the (small) weights are replicated.

Host-side prep (not part of HW exec time):
  - xh    = concat(input, h_prev) along features, transposed to [K, B] fp16
            so the contraction dim K lands on SBUF partitions.
  - W     = concat(W_ih, W_hh) along features, columns reordered so that each
            "quarter" of the gate dim holds a full (i|f|o|g) set for a
            contiguous slice of output dims, then transposed to [K, 4H] fp16.
  - bias  = (b_ih + b_hh), same column reorder, replicated to [128, 4H] fp32.
Device kernel (per core, B_loc = B/8):
  gates[b,g] = xh @ W^T via PE matmul (fp16 in, fp32 PSUM accum) into PSUM
  quarter tiles [128, H]; bias added on the vector engine; sigmoid/tanh on
  the scalar (ACT) engine; c/h elementwise on the vector engine (DVE); fp32
  in/out for c_prev/h_t/c_t.  Quarter 0 runs k-outer over two 4-wide batch
  groups so matmuls start while W streams in; later quarters are prefetched
  (double-buffered W quarter) and run dense per-batch-tile chains.
"""

import numpy as np

import concourse.bacc as bacc
import concourse.mybir as mybir
import concourse.tile as tile
from concourse.bass_utils import run_bass_kernel_spmd

AF = mybir.ActivationFunctionType
F16 = mybir.dt.float16
F32 = mybir.dt.float32

N_CORES = 8
B_TOTAL = 8192
IN_DIM = 1024
H_DIM = 1024
P = 128


def build_lstm_nc(b_loc=B_TOTAL // N_CORES, in_dim=IN_DIM, h_dim=H_DIM,
                  mm_dtype=F16):
    ktot = in_dim + h_dim
    KT = ktot // P              # contraction tiles
    G = 4 * h_dim               # total gate width
    NQ = 4                      # quarters (gate-interleaved column groups)
    QW = G // NQ                # quarter width (== h_dim)
    DS = h_dim // NQ            # output-dim slice per quarter
    NW = min(512, QW)           # matmul moving free width (PSUM bank limit)
    NCH = QW // NW              # matmul chunks per quarter
    BT = b_loc // P             # batch tiles per core
    GRP = min(4, BT)            # batch tiles in flight for k-outer quarter 0

    nc = bacc.Bacc("TRN2", target_bir_lowering=False)
    xhT = nc.dram_tensor("xhT", [ktot, b_loc], mm_dtype, kind="ExternalInput")
    wT = nc.dram_tensor("wT", [ktot, G], mm_dtype, kind="ExternalInput")
    bias = nc.dram_tensor("bias", [P, G], F32, kind="ExternalInput")
    c_prev = nc.dram_tensor("c_prev", [b_loc, h_dim], F32, kind="ExternalInput")
    h_out = nc.dram_tensor("h_out", [b_loc, h_dim], F32, kind="ExternalOutput")
    c_out = nc.dram_tensor("c_out", [b_loc, h_dim], F32, kind="ExternalOutput")

    with tile.TileContext(nc) as tc:
        with (
            tc.tile_pool(name="const", bufs=1) as const_pool,
            tc.tile_pool(name="xh", bufs=1) as xh_pool,
            tc.tile_pool(name="wt", bufs=2) as wt_pool,
            tc.tile_pool(name="work", bufs=3) as work,
            tc.tile_pool(name="psum", bufs=4, space="PSUM") as psum_pool,
        ):
            xh_sb = xh_pool.tile([P, KT * b_loc], mm_dtype)
            wt_tiles = {}

            def load_wt_quarter(q, interleave_xh=False):
                wt_q = wt_pool.tile([P, KT * QW], mm_dtype, name="wt_q")
                wt_tiles[q] = wt_q
                hb = min(GRP * P, b_loc)
                for k in range(KT):
                    if interleave_xh and k == 0:
                        # split the very first transfers so the first batch
                        # group's k0 matmuls unblock before the bulk traffic
                        # piles up on the DMA engines (completion semaphores
                        # fire only when a transfer's last packet drains);
                        # xh's second batch half (group 1, needed ~25us
                        # later) is deferred past k3 to speed k1-k3 arrival
                        nc.sync.dma_start(wt_q[:, 0:NW],
                                          wT[0:P, q * QW:q * QW + NW])
                        nc.sync.dma_start(xh_sb[:, 0:hb], xhT[0:P, 0:hb])
                        if NW < QW:
                            nc.sync.dma_start(
                                wt_q[:, NW:QW],
                                wT[0:P, q * QW + NW:(q + 1) * QW])
                        continue
                    nc.sync.dma_start(
                        wt_q[:, k * QW:(k + 1) * QW],
                        wT[k * P:(k + 1) * P, q * QW:(q + 1) * QW])
                    if interleave_xh:
                        nc.sync.dma_start(
                            xh_sb[:, k * b_loc:(k + 1) * b_loc],
                            xhT[k * P:(k + 1) * P, :])
                        if k == min(3, KT - 1) and hb < b_loc:
                            nc.sync.dma_start(xh_sb[:, hb:b_loc],
                                              xhT[0:P, hb:b_loc])

            # quarter 0 W and the transposed activations, interleaved k-wise
            # so the first accumulation chains can start immediately.
            load_wt_quarter(0, interleave_xh=True)

            # bias loaded per quarter so the 2MB transfer doesn't sit in the
            # DMA queue ahead of quarter 0's c_prev loads
            bias_sb = const_pool.tile([P, G], F32)
            nc.sync.dma_start(bias_sb[:, 0:QW], bias[:, 0:QW])

            # PE warmup: ~3.4us of dummy matmuls on zeroed SBUF while the
            # first W/xh tiles stream in, so HAM is at K=8/8 (2.4 GHz) when
            # real matmuls start.
            scratch = work.tile([P, NW], mm_dtype, name="scratch", bufs=1)
            nc.vector.memset(scratch[:], 0.0)
            zb = const_pool.tile([P, 1], F32)
            nc.vector.memset(zb[:], 0.0)
            ps_w = psum_pool.tile([P, QW], F32, name="ps")
            for i in range(8):
                nc.tensor.matmul(
                    ps_w[:, (i % NCH) * NW:(i % NCH + 1) * NW],
                    scratch[:, 0:P], scratch[:],
                    start=True, stop=True)

            def mm_pair(ps, q, k, b):
                xsl = xh_sb[:, k * b_loc + b * P:k * b_loc + (b + 1) * P]
                wt_q = wt_tiles[q]
                for c in range(NCH):
                    nc.tensor.matmul(
                        ps[:, c * NW:(c + 1) * NW],
                        xsl,
                        wt_q[:, k * QW + c * NW:k * QW + (c + 1) * NW],
                        start=(k == 0), stop=(k == KT - 1))

            def bias_add(ps, q):
                # gates = psum + bias on the DVE. This is the ONLY psum
                # reader, so the PSUM slot frees right after it; emitted for
                # a whole batch group before the rest of the epilogues so the
                # in-order DVE doesn't hold PSUM hostage behind ACT waits.
                gates = work.tile([P, QW], F32, name="gates", bufs=8)
                nc.vector.tensor_add(
                    gates[:], ps[:], bias_sb[:, q * QW:(q + 1) * QW])
                return gates

            def epilogue_tail(gates, q, b):
                # everything past the gate activations; shared with last_tile
                cp = work.tile([P, DS], F32, name="cp")
                nc.sync.dma_start(
                    cp[:], c_prev[b * P:(b + 1) * P, q * DS:(q + 1) * DS])

                ig = work.tile([P, DS], F32, name="ig")
                nc.vector.tensor_mul(ig[:], gates[:, 0:DS],
                                     gates[:, 3 * DS:4 * DS])
                cnew = work.tile([P, DS], F32, name="cnew")
                nc.vector.tensor_mul(cnew[:], gates[:, DS:2 * DS], cp[:])
                nc.vector.tensor_add(cnew[:], cnew[:], ig[:])
                tct = work.tile([P, DS], F32, name="tct")
                nc.scalar.activation(tct[:], cnew[:], AF.Tanh, bias=zb[:])
                hnew = work.tile([P, DS], F32, name="hnew")
                nc.vector.tensor_mul(hnew[:], gates[:, 2 * DS:3 * DS], tct[:])

                nc.sync.dma_start(
                    c_out[b * P:(b + 1) * P, q * DS:(q + 1) * DS], cnew[:])
                nc.sync.dma_start(
                    h_out[b * P:(b + 1) * P, q * DS:(q + 1) * DS], hnew[:])

            def epilogue(gates, q, b):
                # quarter layout: [ i | f | o | g ], each DS wide
                nc.scalar.activation(gates[:, 0:3 * DS], gates[:, 0:3 * DS],
                                     AF.Sigmoid, bias=zb[:])
                nc.scalar.activation(gates[:, 3 * DS:4 * DS],
                                     gates[:, 3 * DS:4 * DS], AF.Tanh,
                                     bias=zb[:])
                epilogue_tail(gates, q, b)

            def last_tile(q, b):
                # Final tile: skew the two 512-wide chunks by LAG k-steps
                # (keeping bank alternation) so the [i|f] half's bias-add and
                # sigmoid overlap the [o|g] half's remaining matmuls.
                LAG = 4
                ps = psum_pool.tile([P, QW], F32, name="ps")
                wt_q = wt_tiles[q]
                for j in range(KT + LAG):
                    for c, k in ((0, j), (1, j - LAG)):
                        if 0 <= k < KT:
                            xsl = xh_sb[:, k * b_loc + b * P:
                                        k * b_loc + (b + 1) * P]
                            nc.tensor.matmul(
                                ps[:, c * NW:(c + 1) * NW],
                                xsl,
                                wt_q[:, k * QW + c * NW:
                                     k * QW + (c + 1) * NW],
                                start=(k == 0), stop=(k == KT - 1))
                gates = work.tile([P, QW], F32, name="gates", bufs=8)
                nc.vector.tensor_add(
                    gates[:, 0:NW], ps[:, 0:NW],
                    bias_sb[:, q * QW:q * QW + NW])
                nc.scalar.activation(gates[:, 0:2 * DS], gates[:, 0:2 * DS],
                                     AF.Sigmoid, bias=zb[:])
                nc.vector.tensor_add(
                    gates[:, NW:2 * NW], ps[:, NW:2 * NW],
                    bias_sb[:, q * QW + NW:q * QW + 2 * NW])
                nc.scalar.activation(gates[:, 2 * DS:3 * DS],
                                     gates[:, 2 * DS:3 * DS],
                                     AF.Sigmoid, bias=zb[:])
                nc.scalar.activation(gates[:, 3 * DS:4 * DS],
                                     gates[:, 3 * DS:4 * DS], AF.Tanh,
                                     bias=zb[:])
                epilogue_tail(gates, q, b)

            # ---- quarter 0: k-outer over GRP-wide batch groups ----
            for g0 in range(0, BT, GRP):
                pss = [psum_pool.tile([P, QW], F32, name="ps")
                       for _ in range(min(GRP, BT - g0))]
                for k in range(KT):
                    for bi, ps in enumerate(pss):
                        mm_pair(ps, 0, k, g0 + bi)
                gts = [bias_add(ps, 0) for ps in pss]
                for bi, gates in enumerate(gts):
                    epilogue(gates, 0, g0 + bi)

            # ---- quarters 1..: prefetched, dense per-b chains ----
            for q in range(1, NQ):
                load_wt_quarter(q)
                nc.sync.dma_start(bias_sb[:, q * QW:(q + 1) * QW],
                                  bias[:, q * QW:(q + 1) * QW])
                for b in range(BT):
                    if q == NQ - 1 and b == BT - 1 and NCH == 2:
                        last_tile(q, b)
                        continue
                    ps = psum_pool.tile([P, QW], F32, name="ps")
                    for k in range(KT):
                        mm_pair(ps, q, k, b)
                    epilogue(bias_add(ps, q), q, b)

    nc.compile()
    return nc


def prep_inputs(input, h_prev, c_prev, W_ih, b_ih, W_hh, b_hh,
                n_cores=N_CORES, np_mm_dtype=np.float16):
    """Host-side shard + layout prep. Returns list of per-core input maps."""
    input = np.asarray(input, np.float32)
    h_prev = np.asarray(h_prev, np.float32)
    c_prev = np.asarray(c_prev, np.float32)
    W_ih = np.asarray(W_ih, np.float32)
    W_hh = np.asarray(W_hh, np.float32)
    b_ih = np.asarray(b_ih, np.float32)
    b_hh = np.asarray(b_hh, np.float32)

    b_total, _ = input.shape
    h_dim = h_prev.shape[1]
    b_loc = b_total // n_cores
    G = 4 * h_dim
    NQ = 4
    DS = h_dim // NQ

    # column reorder: per quarter q the layout is [i | f | o | g] for output
    # dims [q*DS, (q+1)*DS)
    arr = np.arange(G).reshape(4, NQ, DS)       # [gate, q, r]
    idx = arr[[0, 1, 3, 2]].transpose(1, 0, 2).reshape(-1)

    W_cat = np.concatenate([W_ih, W_hh], axis=1)            # [G, ktot]
    wT = np.ascontiguousarray(W_cat[idx, :].T, dtype=np_mm_dtype)
    bias_row = (b_ih + b_hh)[idx].astype(np.float32)
    bias = np.ascontiguousarray(np.broadcast_to(bias_row, (128, G)))

    xh = np.concatenate([input, h_prev], axis=1)            # [B, ktot]
    xhT = xh.T                                              # [ktot, B] (view)

    in_maps = []
    for c in range(n_cores):
        in_maps.append({
            "xhT": np.ascontiguousarray(
                xhT[:, c * b_loc:(c + 1) * b_loc], dtype=np_mm_dtype),
            "wT": wT,
            "bias": bias,
            "c_prev": np.ascontiguousarray(c_prev[c * b_loc:(c + 1) * b_loc]),
        })
    return in_maps


def run_lstm(inputs, trace=False, **spmd_kwargs):
    """Builds + runs the kernel on all 8 cores. Returns (h_t, c_t), results."""
    in_maps = prep_inputs(**inputs)
    nc = build_lstm_nc()
    res = run_bass_kernel_spmd(nc, in_maps, core_ids=list(range(N_CORES)),
                               trace=trace, **spmd_kwargs)
    h_t = np.concatenate([r["h_out"] for r in res.results], axis=0)
    c_t = np.concatenate([r["c_out"] for r in res.results], axis=0)
    return (h_t, c_t), res


def kernel(input, h_prev, c_prev, W_ih, b_ih, W_hh, b_hh):
    (h_t, c_t), _ = run_lstm(dict(
        input=input, h_prev=h_prev, c_prev=c_prev,
        W_ih=W_ih, b_ih=b_ih, W_hh=W_hh, b_hh=b_hh))
    return (h_t, c_t)



# revision 2
# speedup vs baseline: 1.3976x; 1.3976x over previous
"""DPLSTMCell Trainium2 kernel — per-gate mixed precision (fp8 + fp16).

Data-parallel LSTM cell over 8 NeuronCores: batch dim of input/h_prev/c_prev
is sharded, the (small) weights are replicated.

Precision scheme (error budget rel<2e-2; measured rel_h≈1.6e-2):
  The four gate pre-activations have very different sensitivity to fp8
  quantization noise (h-error when ONLY that gate is fp8):
      i: 0.62e-2   f: 0.89e-2   o: 1.21e-2   g: 2.00e-2
  so gates i,f,o use fp8e4m3 DoubleRow matmuls (2 k-rows/cycle, 2x fp16
  throughput) while the tanh-gate g stays fp16.  Errors add in quadrature:
  sqrt(.62^2+.89^2+1.21^2) = 1.62e-2 < 2e-2.  PE work drops from 1024 to
  768 x 512-row-equivalents per core (~138us vs ~221us roofline).

  Both operands are pre-scaled host-side by powers of two (x*32, W*4096,
  exact in fp16) so fp8 values avoid the subnormal range; the whole PSUM
  is then uniformly scaled by 2^17 and descaled for free by the ACT
  engine's `scale` immediate: sigmoid(2^-17 * (psum + bias*2^17)).

Host-side prep (not part of HW exec time):
  - columns of W reordered so each 1024-wide "quarter" holds a full
    [i|f|o|g] set (256 each) for a contiguous slice of output dims; fp8
    blocks (i,f,o = 768 cols) and fp16 block (g = 256 cols) are packed
    into PE-ready DoubleRow / k-tile layouts with the contraction dim on
    SBUF partitions.
  - xh = concat(input, h_prev): quantized once to fp8 (DoubleRow pair
    layout) and once to fp16, batch-tile-major so per-b-tile DMAs land
    in compute order.
Device kernel (per core, B_loc = B/8 = 1024):
  per (quarter q, batch-tile b): PSUM tile [128,1024] accumulates
    cols 0:512   (i|f): 8 fp8 DoubleRow matmuls over K=2048
    cols 512:768 (o):   8 fp8 DoubleRow matmuls
    cols 768:1024(g):  16 fp16 matmuls
  DVE adds the (pre-scaled) bias, ACT applies sigmoid/tanh with
  scale=2^-17, then the usual c/h elementwise tail on DVE + ACT.
"""

import numpy as np
import ml_dtypes

import concourse.bacc as bacc
import concourse.mybir as mybir
import concourse.tile as tile
from concourse.bass_utils import run_bass_kernel_spmd

AF = mybir.ActivationFunctionType
DR = mybir.MatmulPerfMode.DoubleRow
F8 = mybir.dt.float8e4
F16 = mybir.dt.float16
F32 = mybir.dt.float32

N_CORES = 8
B_TOTAL = 8192
IN_DIM = 1024
H_DIM = 1024
P = 128

SX = 32.0        # x pre-scale (power of two)
SW = 4096.0      # W pre-scale (power of two)
INV = 1.0 / (SX * SW)   # 2^-17, exact


def build_lstm_nc(b_loc=B_TOTAL // N_CORES, in_dim=IN_DIM, h_dim=H_DIM):
    ktot = in_dim + h_dim
    KT16 = ktot // P            # fp16 k-tiles (g gate)
    KT8 = ktot // (2 * P)       # fp8 DoubleRow k-super-tiles (i,f,o gates)
    G = 4 * h_dim               # total gate width
    NQ = 4                      # quarters, each [i|f|o|g] x DS
    QW = G // NQ                # quarter width (1024)
    DS = h_dim // NQ            # output-dim slice per quarter (256)
    W8C = 3 * DS                # fp8 cols per quarter (768: i,f,o)
    BT = b_loc // P             # batch tiles per core (8)
    GRP = min(4, BT)            # batch tiles in flight for quarter 0

    nc = bacc.Bacc("TRN2", target_bir_lowering=False)
    # PE-ready host layouts; leading dim = SBUF partition (contraction k%128)
    xh16 = nc.dram_tensor("xh16", [P, BT, KT16, P], F16, kind="ExternalInput")
    xh8 = nc.dram_tensor("xh8", [P, BT, KT8, 2, P], F8, kind="ExternalInput")
    w16 = nc.dram_tensor("w16", [NQ, P, KT16, DS], F16, kind="ExternalInput")
    w8 = nc.dram_tensor("w8", [NQ, P, KT8, 2, W8C], F8, kind="ExternalInput")
    bias = nc.dram_tensor("bias", [P, G], F32, kind="ExternalInput")
    c_prev = nc.dram_tensor("c_prev", [b_loc, h_dim], F32, kind="ExternalInput")
    h_out = nc.dram_tensor("h_out", [b_loc, h_dim], F32, kind="ExternalOutput")
    c_out = nc.dram_tensor("c_out", [b_loc, h_dim], F32, kind="ExternalOutput")

    with tile.TileContext(nc) as tc:
        with (
            tc.tile_pool(name="const", bufs=1) as const_pool,
            tc.tile_pool(name="xh", bufs=1) as xh_pool,
            tc.tile_pool(name="w8p", bufs=2) as w8_pool,
            tc.tile_pool(name="w16p", bufs=2) as w16_pool,
            tc.tile_pool(name="work", bufs=3) as work,
            tc.tile_pool(name="psum", bufs=4, space="PSUM") as psum_pool,
        ):
            xh16_sb = xh_pool.tile([P, BT, KT16, P], F16)
            xh8_sb = xh_pool.tile([P, BT, KT8, 2, P], F8)
            bias_sb = const_pool.tile([P, G], F32)
            w8_tiles = {}
            w16_tiles = {}

            def load_w_quarter(q, split=False):
                w8_q = w8_pool.tile([P, KT8, 2, W8C], F8, name="w8q")
                w16_q = w16_pool.tile([P, KT16, DS], F16, name="w16q")
                w8_tiles[q] = w8_q
                w16_tiles[q] = w16_q
                if not split:
                    nc.sync.dma_start(w8_q[:], w8[q, :, :, :, :])
                    nc.sync.dma_start(w16_q[:], w16[q, :, :, :])
                    nc.sync.dma_start(bias_sb[:, q * QW:(q + 1) * QW],
                                      bias[:, q * QW:(q + 1) * QW])
                    return
                # quarter 0: split + interleave with xh so the first batch
                # group's chains unblock as early as possible (DMA completion
                # semaphores fire only when a transfer's last packet drains).
                nc.sync.dma_start(w8_q[:, 0, :, :], w8[q, :, 0, :, :])
                for b in range(GRP):
                    nc.sync.dma_start(xh8_sb[:, b], xh8[:, b])
                for t in range(1, KT8):
                    nc.sync.dma_start(w8_q[:, t, :, :], w8[q, :, t, :, :])
                for k in range(0, KT16, 4):
                    nc.sync.dma_start(w16_q[:, k:k + 4, :],
                                      w16[q, :, k:k + 4, :])
                for b in range(GRP):
                    nc.sync.dma_start(xh16_sb[:, b], xh16[:, b])
                nc.sync.dma_start(bias_sb[:, q * QW:(q + 1) * QW],
                                  bias[:, q * QW:(q + 1) * QW])
                for b in range(GRP, BT):
                    nc.sync.dma_start(xh8_sb[:, b], xh8[:, b])
                    nc.sync.dma_start(xh16_sb[:, b], xh16[:, b])

            load_w_quarter(0, split=True)

            # PE warmup: dummy matmuls on zeroed SBUF while the first W/xh
            # tiles stream in, so the PE p-state is at full clock when real
            # matmuls start.
            scratch = work.tile([P, 512], F16, name="scratch", bufs=1)
            nc.vector.memset(scratch[:], 0.0)
            zb = const_pool.tile([P, 1], F32)
            nc.vector.memset(zb[:], 0.0)
            ps_w = psum_pool.tile([P, QW], F32, name="ps")
            for i in range(8):
                nc.tensor.matmul(
                    ps_w[:, (i % 2) * 512:(i % 2) * 512 + 512],
                    scratch[:, 0:P], scratch[:],
                    start=True, stop=True)

            def mm_fp8(ps, q, t, b):
                # i|f chunk (cols 0:512, psum bank A) and o chunk (512:768)
                w8_q = w8_tiles[q]
                xsl = xh8_sb[:, b, t, :, :]
                nc.tensor.matmul(ps[:, 0:512], xsl, w8_q[:, t, :, 0:512],
                                 perf_mode=DR,
                                 start=(t == 0), stop=(t == KT8 - 1))
                nc.tensor.matmul(ps[:, 512:W8C], xsl, w8_q[:, t, :, 512:W8C],
                                 perf_mode=DR,
                                 start=(t == 0), stop=(t == KT8 - 1))

            def mm_fp16(ps, q, k, b):
                # g chunk (cols 768:1024, psum bank B)
                nc.tensor.matmul(ps[:, W8C:QW],
                                 xh16_sb[:, b, k, :],
                                 w16_tiles[q][:, k, :],
                                 start=(k == 0), stop=(k == KT16 - 1))

            def bias_add(ps, q):
                # gates_scaled = psum + bias*2^17 on the DVE; the ONLY psum
                # reader, so the PSUM slot frees right after it.
                gates = work.tile([P, QW], F32, name="gates", bufs=8)
                nc.vector.tensor_add(
                    gates[:], ps[:], bias_sb[:, q * QW:(q + 1) * QW])
                return gates

            def epilogue(gates, q, b):
                # quarter layout: [ i | f | o | g ], each DS wide; ACT
                # descales by 2^-17 via its scale immediate.
                nc.scalar.activation(gates[:, 0:3 * DS], gates[:, 0:3 * DS],
                                     AF.Sigmoid, bias=zb[:], scale=INV)
                nc.scalar.activation(gates[:, 3 * DS:4 * DS],
                                     gates[:, 3 * DS:4 * DS], AF.Tanh,
                                     bias=zb[:], scale=INV)
                cp = work.tile([P, DS], F32, name="cp")
                nc.sync.dma_start(
                    cp[:], c_prev[b * P:(b + 1) * P, q * DS:(q + 1) * DS])

                ig = work.tile([P, DS], F32, name="ig")
                nc.vector.tensor_mul(ig[:], gates[:, 0:DS],
                                     gates[:, 3 * DS:4 * DS])
                cnew = work.tile([P, DS], F32, name="cnew")
                nc.vector.tensor_mul(cnew[:], gates[:, DS:2 * DS], cp[:])
                nc.vector.tensor_add(cnew[:], cnew[:], ig[:])
                tct = work.tile([P, DS], F32, name="tct")
                nc.scalar.activation(tct[:], cnew[:], AF.Tanh, bias=zb[:])
                hnew = work.tile([P, DS], F32, name="hnew")
                nc.vector.tensor_mul(hnew[:], gates[:, 2 * DS:3 * DS], tct[:])

                nc.sync.dma_start(
                    c_out[b * P:(b + 1) * P, q * DS:(q + 1) * DS], cnew[:])
                nc.sync.dma_start(
                    h_out[b * P:(b + 1) * P, q * DS:(q + 1) * DS], hnew[:])

            # ---- quarter 0: k-outer over GRP-wide batch groups so matmuls
            # start while W/xh stream in ----
            for g0 in range(0, BT, GRP):
                nb = min(GRP, BT - g0)
                pss = [psum_pool.tile([P, QW], F32, name="ps")
                       for _ in range(nb)]
                for t in range(KT8):
                    for bi, ps in enumerate(pss):
                        mm_fp8(ps, 0, t, g0 + bi)
                for k in range(KT16):
                    for bi, ps in enumerate(pss):
                        mm_fp16(ps, 0, k, g0 + bi)
                gts = [bias_add(ps, 0) for ps in pss]
                for bi, gates in enumerate(gts):
                    epilogue(gates, 0, g0 + bi)

            # ---- quarters 1..3: prefetched, dense per-b chains ----
            for q in range(1, NQ):
                load_w_quarter(q)
                for b in range(BT):
                    ps = psum_pool.tile([P, QW], F32, name="ps")
                    for t in range(KT8):
                        mm_fp8(ps, q, t, b)
                    for k in range(KT16):
                        mm_fp16(ps, q, k, b)
                    epilogue(bias_add(ps, q), q, b)

    nc.compile()
    return nc


def prep_inputs(input, h_prev, c_prev, W_ih, b_ih, W_hh, b_hh,
                n_cores=N_CORES):
    """Host-side shard + quantize + layout prep. Per-core input maps."""
    input = np.asarray(input, np.float32)
    h_prev = np.asarray(h_prev, np.float32)
    c_prev = np.asarray(c_prev, np.float32)
    W_ih = np.asarray(W_ih, np.float32)
    W_hh = np.asarray(W_hh, np.float32)
    b_ih = np.asarray(b_ih, np.float32)
    b_hh = np.asarray(b_hh, np.float32)

    b_total, in_dim = input.shape
    h_dim = h_prev.shape[1]
    ktot = in_dim + h_dim
    b_loc = b_total // n_cores
    G = 4 * h_dim
    NQ = 4
    DS = h_dim // NQ
    W8C = 3 * DS
    BT = b_loc // 128
    KT16 = ktot // 128
    KT8 = ktot // 256

    def q8(x):
        return np.clip(x, -240, 240).astype(ml_dtypes.float8_e4m3)

    # column reorder: per quarter q the layout is [i | f | o | g] for output
    # dims [q*DS, (q+1)*DS)
    arr = np.arange(G).reshape(4, NQ, DS)       # [gate, q, r]
    idx = arr[[0, 1, 3, 2]].transpose(1, 0, 2).reshape(-1)

    W_cat = np.concatenate([W_ih, W_hh], axis=1)[idx, :]    # [G, ktot] scaled
    Ws = W_cat * SW
    # fp8 blocks (i,f,o = first 768 cols of each quarter) in DoubleRow layout
    w8_host = np.empty((NQ, 128, KT8, 2, W8C), ml_dtypes.float8_e4m3)
    w16_host = np.empty((NQ, 128, KT16, DS), np.float16)
    for q in range(NQ):
        blk8 = q8(Ws[q * 1024:q * 1024 + W8C, :]).T         # [ktot, 768]
        w8_host[q] = blk8.reshape(KT8, 2, 128, W8C).transpose(2, 0, 1, 3)
        blk16 = Ws[q * 1024 + W8C:(q + 1) * 1024, :].T.astype(np.float16)
        w16_host[q] = blk16.reshape(KT16, 128, DS).transpose(1, 0, 2)

    bias_row = ((b_ih + b_hh)[idx] * (SX * SW)).astype(np.float32)
    bias = np.ascontiguousarray(np.broadcast_to(bias_row, (128, G)))

    xh = np.concatenate([input, h_prev], axis=1) * SX       # [B, ktot] scaled
    x8 = q8(xh)
    x16 = xh.astype(np.float16)

    in_maps = []
    for c in range(n_cores):
        rows = slice(c * b_loc, (c + 1) * b_loc)
        xc8 = x8[rows].T                                    # [ktot, b_loc]
        xc16 = x16[rows].T
        # [p, b, t, s, m] = x[t*256 + s*128 + p, b*128 + m]
        xh8_h = xc8.reshape(KT8, 2, 128, BT, 128).transpose(2, 3, 0, 1, 4)
        xh16_h = xc16.reshape(KT16, 128, BT, 128).transpose(1, 2, 0, 3)
        in_maps.append({
            "xh8": np.ascontiguousarray(xh8_h),
            "xh16": np.ascontiguousarray(xh16_h),
            "w8": w8_host,
            "w16": w16_host,
            "bias": bias,
            "c_prev": np.ascontiguousarray(c_prev[rows]),
        })
    return in_maps


def run_lstm(inputs, trace=False, **spmd_kwargs):
    """Builds + runs the kernel on all 8 cores. Returns (h_t, c_t), results."""
    in_maps = prep_inputs(**inputs)
    nc = build_lstm_nc()
    res = run_bass_kernel_spmd(nc, in_maps, core_ids=list(range(N_CORES)),
                               trace=trace, **spmd_kwargs)
    h_t = np.concatenate([r["h_out"] for r in res.results], axis=0)
    c_t = np.concatenate([r["c_out"] for r in res.results], axis=0)
    return (h_t, c_t), res


def kernel(input, h_prev, c_prev, W_ih, b_ih, W_hh, b_hh):
    (h_t, c_t), _ = run_lstm(dict(
        input=input, h_prev=h_prev, c_prev=c_prev,
        W_ih=W_ih, b_ih=b_ih, W_hh=W_hh, b_hh=b_hh))
    return (h_t, c_t)
